# revision 39
# baseline (speedup 1.0000x reference)
"""ExternalAttention (BN + external-attention) Trainium2 Bass kernel.

Full-input contract: kernel(**inputs) takes the unsharded inputs and
returns the full output. Internally shards batch B=8 across 8 NeuronCores
(data parallel); BN batch stats are combined with a 4KB AllGather.

Math notes:
  - softmax over spatial positions is invariant to per-(b,i) additive
    constants, so beta and the BN mean-shift drop out of the q path;
    only s[c] = gamma[c] * rsqrt(var[c] + eps) is needed, folded into kT.
  - the +1e-6 in the head-channel L1 norm shifts r by ~1e-4 relative
    (s ~ 7.8e-3) - far below the bf16 noise floor used downstream, so it
    is folded in via the reciprocal input bias path (add) when cheap.
"""
import numpy as np
import ml_dtypes

import concourse.bass as bass
import concourse.tile as tile
from concourse import bacc, mybir
from concourse.bass_utils import run_bass_kernel_spmd

N_CORES = 8
B, C_IN, H, W = 8, 512, 64, 64
HW = H * W                      # 4096
C_INTER, C_OUT = 256, 512
NUM_HEADS = 8
DH = C_INTER // NUM_HEADS       # 32
BN_EPS = 1e-5
NT = HW // 512                  # 8 spatial tiles of 512
PC = C_IN // 128                # 4 channel chunks
IH = C_INTER // 128             # 2 i-halves
OQ = C_OUT // 128               # 4 output quarters

F32 = mybir.dt.float32
F32R = mybir.dt.float32r
BF16 = mybir.dt.bfloat16


def build_kernel(n_cores=N_CORES, with_collective=True):
    nc = bacc.Bacc("TRN2", target_bir_lowering=False, debug=False,
                   num_devices=n_cores)
    x_d = nc.dram_tensor("x", [C_IN, HW], F32, kind="ExternalInput").ap()
    kt_d = nc.dram_tensor("kT", [C_IN, C_INTER], F32, kind="ExternalInput").ap()
    vt_d = nc.dram_tensor("vT", [C_INTER, C_OUT], F32, kind="ExternalInput").ap()
    g_d = nc.dram_tensor("gamma", [PC, 128, 1], F32, kind="ExternalInput").ap()
    mh_d = nc.dram_tensor("maskh", [128, 4], BF16, kind="ExternalInput").ap()
    mw_d = nc.dram_tensor("maskw", [16, NT * 128], BF16,
                          kind="ExternalInput").ap()
    mp_d = nc.dram_tensor("maskp", [128, NT * 32], BF16,
                          kind="ExternalInput").ap()
    out_d = nc.dram_tensor("out", [C_OUT, HW], F32, kind="ExternalOutput").ap()

    with tile.TileContext(nc) as tc:
        with (
            tc.tile_pool(name="px", bufs=PC) as px,
            tc.tile_pool(name="psm", bufs=1) as psm,          # small singles
            tc.tile_pool(name="pstat", bufs=PC) as pstat,
            tc.tile_pool(name="pe", bufs=IH) as pe_pool,      # exp values
            tc.tile_pool(name="pr", bufs=4) as pr_pool,       # r tiles
            tc.tile_pool(name="po", bufs=3) as po_pool,       # out staging
            tc.tile_pool(name="pz", bufs=IH) as pz_pool,
            tc.tile_pool(name="dram", bufs=1, space="DRAM") as dram,
            tc.tile_pool(name="ps_q", bufs=2, space="PSUM") as ps_q,
            tc.tile_pool(name="ps_s", bufs=2, space="PSUM") as ps_s,
            tc.tile_pool(name="ps_w", bufs=2, space="PSUM") as ps_w,
            tc.tile_pool(name="ps_o", bufs=2, space="PSUM") as ps_o,
        ):
            # ---- load weights / constants ----
            eps_t = psm.tile([128, 1], F32, tag="eps")
            nc.vector.memset(eps_t, BN_EPS)
            # prefetch the Sqrt and Exp ACT tables off the critical path
            actwarm = psm.tile([128, 1], F32, tag="actwarm")
            nc.scalar.activation(out=actwarm, in_=eps_t,
                                 func=mybir.ActivationFunctionType.Sqrt)
            expwarm = psm.tile([128, 1], F32, tag="expwarm")
            nc.scalar.activation(out=expwarm, in_=eps_t,
                                 func=mybir.ActivationFunctionType.Exp)

            # ---- load x (as f32r for matmul1) + local BN partial stats ----
            # x per chunk in two half DMAs so bn_stats starts on the first
            # half early; stats AllGather'd per chunk-PAIR so the first
            # collective overlaps the remaining x loads.
            xs = []
            stats_all = psm.tile([128, 2 * PC], F32, tag="stats_all")
            for c in range(PC):
                x_c = px.tile([128, HW], F32R, tag="x")
                xd = x_d[c * 128:(c + 1) * 128, :].bitcast(F32R)
                # last chunk in quarters: its bn_stats tail gates the
                # AllGather, so start stats on earlier pieces sooner
                nsplit = 4 if c == PC - 1 else 2
                step = HW // nsplit
                for sp in range(nsplit):
                    nc.sync.dma_start(
                        out=x_c[:, sp * step:(sp + 1) * step],
                        in_=xd[:, sp * step:(sp + 1) * step])
                xs.append(x_c)
                st6 = pstat.tile([128, NT, 6], F32, tag="st6")
                xv = x_c.bitcast(F32)
                for j in range(NT):
                    nc.vector.bn_stats(
                        out=st6[:, j, :], in_=xv[:, j * 512:(j + 1) * 512])
                mv = pstat.tile([128, 2], F32, tag="mv")
                nc.vector.bn_aggr(out=mv, in_=st6)
                # partial = (mean, E[x^2]) = (mean, var + mean^2)
                nc.vector.tensor_copy(
                    out=stats_all[:, 2 * c:2 * c + 1], in_=mv[:, 0:1])
                msq = pstat.tile([128, 1], F32, tag="msq")
                nc.vector.tensor_mul(out=msq, in0=mv[:, 0:1], in1=mv[:, 0:1])
                nc.vector.tensor_add(
                    out=stats_all[:, 2 * c + 1:2 * c + 2],
                    in0=msq, in1=mv[:, 1:2])

            maskh = psm.tile([128, 4], BF16, tag="maskh")
            nc.sync.dma_start(out=maskh, in_=mh_d)
            maskw = psm.tile([16, NT * 128], BF16, tag="maskw")
            nc.sync.dma_start(out=maskw, in_=mw_d)
            mpk = psm.tile([128, NT * 32], BF16, tag="maskp")
            nc.sync.dma_start(out=mpk, in_=mp_d)
            maskp_t = [mpk[:, n * 32:(n + 1) * 32] for n in range(NT)]
            kts = []
            for c in range(PC):
                kt_c = psm.tile([128, C_INTER], F32, tag=f"kt{c}")
                nc.sync.dma_start(out=kt_c, in_=kt_d[c * 128:(c + 1) * 128, :])
                kts.append(kt_c)
            gamma_all = psm.tile([128, PC], F32, tag="gamma_all")
            nc.sync.dma_start(out=gamma_all,
                              in_=g_d.rearrange("c p o -> p (c o)"))

            vtbf = []
            for ic in range(IH):
                vt_c = psm.tile([128, C_OUT], F32, tag=f"vt{ic}")
                nc.sync.dma_start(out=vt_c, in_=vt_d[ic * 128:(ic + 1) * 128, :])
                vb = psm.tile([128, C_OUT], BF16, tag=f"vtb{ic}")
                nc.scalar.copy(out=vb, in_=vt_c)
                vtbf.append(vb)

            # ---- AllGather partial stats, combine locally ----
            ag_in = dram.tile([128, 2 * PC], F32, tag="agi")
            ag_out = dram.tile([N_CORES, 128, 2 * PC], F32, tag="ago")
            nc.sync.dma_start(out=ag_in, in_=stats_all)
            if with_collective:
                nc.gpsimd.collective_compute(
                    "AllGather",
                    mybir.AluOpType.bypass,
                    replica_groups=[list(range(N_CORES))],
                    ins=[ag_in.opt()],
                    outs=[ag_out.opt()],
                )
            else:
                # sim-only stand-in: one broadcast DMA (real AG floor ~5us)
                nc.sync.dma_start(
                    out=ag_out,
                    in_=ag_in.unsqueeze(0).broadcast_to(
                        [N_CORES, 128, 2 * PC]))
            g_all = psm.tile([128, N_CORES, 2 * PC], F32, tag="g_all")
            nc.sync.dma_start(out=g_all,
                              in_=ag_out.rearrange("r p s -> p r s"))

            # PE warmup during the collective window: junk matmuls gated on
            # stats_all so they land in the otherwise-idle gap and lift the
            # HAM clock before matmul1 starts.
            junk = psm.tile([128, 512], BF16, tag="junk")
            nc.vector.memset(junk, 0.5)
            jdep = psm.tile([128, 8], BF16, tag="jdep")
            nc.vector.tensor_copy(out=jdep, in_=stats_all)
            nc.vector.tensor_copy(out=junk[:, 0:8], in_=jdep)
            for j in range(12):
                wm = ps_s.tile([4, 512], F32, tag="ps")
                nc.tensor.matmul(wm, lhsT=maskh, rhs=junk,
                                 start=True, stop=True)
            # second warmup batch gated on the gathered stats: fills the
            # post-AllGather chain window so matmul1 starts at full clock
            jdep2 = psm.tile([128, 8], BF16, tag="jdep2")
            nc.vector.tensor_copy(out=jdep2, in_=g_all[:, 0, :])
            nc.vector.tensor_copy(out=junk[:, 8:16], in_=jdep2)
            for j in range(8):
                wm = ps_s.tile([4, 512], F32, tag="ps")
                nc.tensor.matmul(wm, lhsT=maskh, rhs=junk,
                                 start=True, stop=True)

            # batched: global mean/var -> s = gamma*rsqrt(var+eps), k' = kT*s
            tot8 = pstat.tile([128, 2 * PC], F32, tag="tot8")
            nc.vector.tensor_reduce(
                out=tot8, in_=g_all.rearrange("p r s -> p s r"),
                axis=mybir.AxisListType.X, op=mybir.AluOpType.add)
            nc.vector.tensor_scalar_mul(tot8, tot8, 1.0 / N_CORES)
            pairs = tot8.rearrange("p (c s) -> p s c", s=2)
            meanv, ex2v = pairs[:, 0, :], pairs[:, 1, :]     # [128, PC]
            varg = pstat.tile([128, PC], F32, tag="varg")
            nc.vector.tensor_mul(out=varg, in0=meanv, in1=meanv)
            nc.vector.tensor_sub(out=varg, in0=ex2v, in1=varg)
            sd = pstat.tile([128, PC], F32, tag="sd")
            nc.scalar.activation(
                out=sd, in_=varg,
                func=mybir.ActivationFunctionType.Sqrt, bias=eps_t)
            expwarm2 = psm.tile([128, PC], F32, tag="expwarm2")
            nc.scalar.activation(out=expwarm2, in_=sd,
                                 func=mybir.ActivationFunctionType.Exp)
            s_all = pstat.tile([128, PC], F32, tag="s_all")
            nc.vector.reciprocal(out=s_all, in_=sd)
            nc.vector.tensor_mul(out=s_all, in0=s_all, in1=gamma_all)
            krs = []
            for c in range(PC):
                kr_c = psm.tile([128, C_INTER], F32R, tag=f"kr{c}")
                nc.vector.tensor_scalar_mul(kr_c, kts[c], s_all[:, c:c + 1])
                krs.append(kr_c)

            # ---- matmul1 + softmax + head-norm per i-half ----
            # e is split into two [128, HW/2] tiles per half so matmul2 can
            # start on the first spatial half while the second is finishing.
            # Emission order is hand-interleaved so h1's tiny Z/rz/t ops are
            # not queued on DVE behind all of h0's qf multiplies (per-engine
            # scheduling order follows code order).
            HH = HW // 2

            def alloc_e(h):
                eg = []
                for g in range(2):
                    e_g = pe_pool.tile([128, HH], BF16, tag=f"e{h}{g}",
                                       bufs=1)
                    eg.append(e_g)
                zp = pz_pool.tile([128, NT], F32, tag=f"zp{h}", bufs=1)
                return eg, zp

            def emit_mm1_exp(h, eg, zp, n_list):
                for n in n_list:
                    pq = ps_q.tile([128, 512], F32, tag="pq")
                    for c in range(PC):
                        nc.tensor.matmul(
                            pq,
                            lhsT=krs[c][:, h * 128:(h + 1) * 128],
                            rhs=xs[c][:, n * 512:(n + 1) * 512],
                            start=(c == 0), stop=(c == PC - 1))
                    g, nn = divmod(n, NT // 2)
                    nc.scalar.activation(
                        out=eg[g][:, nn * 512:(nn + 1) * 512], in_=pq,
                        func=mybir.ActivationFunctionType.Exp,
                        accum_out=zp[:, n:n + 1])

            def emit_z_t(eg, zp):
                z_h = pz_pool.tile([128, 1], F32, tag="z")
                nc.vector.tensor_reduce(
                    out=z_h, in_=zp, axis=mybir.AxisListType.X,
                    op=mybir.AluOpType.add)
                rz = pz_pool.tile([128, 1], F32, tag="rz")
                nc.vector.reciprocal(out=rz, in_=z_h)
                # t = e / Z, in place (bf16 fast mode)
                nc.vector.tensor_scalar_mul(eg[0], eg[0], rz)
                nc.vector.tensor_scalar_mul(eg[1], eg[1], rz)

            def emit_headsum(eg, half):
                # head-sums for one spatial half packed into one PSUM bank
                # (tile nn -> partitions 4nn..4nn+3) by accumulating M=16
                # matmuls whose mask variant is zero outside column block
                # nn; one reciprocal covers the half so the first w/qf can
                # start after 4 s-matmuls instead of 8.
                ps = ps_s.tile([16, 512], F32, tag="ps", bufs=2)
                for nn in range(NT // 2):
                    nc.tensor.matmul(
                        ps, lhsT=maskp_t[nn][:, 0:16],
                        rhs=eg[half][:, nn * 512:(nn + 1) * 512],
                        start=(nn == 0), stop=(nn == NT // 2 - 1))
                r_pk = pr_pool.tile([16, 512], BF16, tag="r")
                with nc.allow_low_precision("head-sum recip to bf16; "
                                            "0.4% well under tolerance"):
                    nc.vector.reciprocal(out=r_pk, in_=ps)
                return r_pk

            def emit_w_qf(eg, r_pks, n_list):
                for n in n_list:
                    g, nn = divmod(n, NT // 2)
                    ns = slice(nn * 512, (nn + 1) * 512)
                    pw = ps_w.tile([128, 512], F32, tag="pw")
                    nc.tensor.matmul(
                        pw, lhsT=maskw[:, n * 128:(n + 1) * 128],
                        rhs=r_pks[g], start=True, stop=True)
                    nc.vector.tensor_mul(out=eg[g][:, ns], in0=eg[g][:, ns],
                                         in1=pw)

            eg0, zp0 = alloc_e(0)
            eg1, zp1 = alloc_e(1)
            emit_mm1_exp(0, eg0, zp0, range(NT))
            emit_z_t(eg0, zp0)
            r0 = [emit_headsum(eg0, 0), emit_headsum(eg0, 1)]
            emit_w_qf(eg0, r0, range(NT))
            emit_mm1_exp(1, eg1, zp1, range(NT))
            emit_z_t(eg1, zp1)
            r1 = [emit_headsum(eg1, 0), emit_headsum(eg1, 1)]
            emit_w_qf(eg1, r1, range(NT))
            ts = [eg0, eg1]

            # ---- matmul2: out = vT.T @ qf ----
            # half-major so the output DMA stream starts as soon as the
            # first spatial half of a quarter is evacuated
            for half in range(2):
                for oq in range(OQ):
                    ost = po_pool.tile([128, HH], F32, tag=f"ost{half}")
                    first = (half == 0 and oq == 0)
                    for nn in range(NT // 2):
                        ns = slice(nn * 512, (nn + 1) * 512)
                        po = ps_o.tile([128, 512], F32, tag="po")
                        for ic in range(IH):
                            nc.tensor.matmul(
                                po,
                                lhsT=vtbf[ic][:, oq * 128:(oq + 1) * 128],
                                rhs=ts[ic][half][:, ns],
                                start=(ic == 0), stop=(ic == IH - 1))
                        nc.scalar.copy(
                            out=ost[:, ns], in_=po)
                        if first and nn == 1:
                            # start the output stream as early as possible
                            nc.sync.dma_start(
                                out=out_d[0:128, 0:1024], in_=ost[:, 0:1024])
                    if first:
                        nc.sync.dma_start(
                            out=out_d[0:128, 1024:2048], in_=ost[:, 1024:2048])
                    else:
                        nc.sync.dma_start(
                            out=out_d[oq * 128:(oq + 1) * 128,
                                      half * HH:(half + 1) * HH],
                            in_=ost)

    nc.compile()
    return nc


_NC_CACHE = None


def _get_nc():
    global _NC_CACHE
    if _NC_CACHE is None:
        _NC_CACHE = build_kernel()
    return _NC_CACHE


def _make_masks():
    mh = np.zeros((128, 4), dtype=ml_dtypes.bfloat16)
    for p in range(128):
        mh[p, p // DH] = 1
    # w-broadcast selector variants: lhsT_n[k, i] = 1 iff k == 4n + i//DH,
    # so rhs can be the full packed r (base partition 0)
    mw = np.zeros((16, NT * 128), dtype=ml_dtypes.bfloat16)
    for n in range(NT):
        for i in range(128):
            mw[4 * (n % 4) + i // DH, n * 128 + i] = 1
    # 8 shifted variants for the packed head-sum matmul: variant n is
    # [128, 32] with the (p -> 4n + p//32) block set, zero elsewhere
    mp = np.zeros((128, NT * 32), dtype=ml_dtypes.bfloat16)
    for n in range(NT):
        for p in range(128):
            mp[p, n * 32 + 4 * n + p // DH] = 1
    return mh, mw, mp


def make_in_maps(x, k, v, gamma):
    mh, mw, mp = _make_masks()
    kt = np.ascontiguousarray(k.T)                    # [C_IN, C_INTER]
    vt = np.ascontiguousarray(v.T)                    # [C_INTER, C_OUT]
    g4 = np.ascontiguousarray(
        gamma.reshape(PC, 128, 1).astype(np.float32))
    in_maps = []
    for i in range(N_CORES):
        in_maps.append({
            "x": np.ascontiguousarray(x[i].reshape(C_IN, HW)),
            "kT": kt, "vT": vt, "gamma": g4,
            "maskh": mh, "maskw": mw, "maskp": mp,
        })
    return in_maps


def kernel(x, k, v, gamma, beta):
    assert x.shape == (B, C_IN, H, W)
    nc = _get_nc()
    in_maps = make_in_maps(np.asarray(x), np.asarray(k), np.asarray(v),
                           np.asarray(gamma))
    try:
        res = run_bass_kernel_spmd(nc, in_maps, list(range(N_CORES)))
    except Exception:
        # one retry after clearing jax caches (rare one-off flake where a
        # stale trace cache leaves two bass_exec calls in one XLA module)
        import jax
        jax.clear_caches()
        res = run_bass_kernel_spmd(nc, in_maps, list(range(N_CORES)))
    out = np.stack([res.results[i]["out"].reshape(C_OUT, H, W)
                    for i in range(N_CORES)])
    return out.astype(np.float32)
